# revision 16
# baseline (speedup 1.0000x reference)
"""Multi-head attention Trainium2 kernel (Bass/Tile), 8-core SPMD.

Problem: nn.MultiHeadAttention, B=2, S=2048, D=1024, H=16 heads, DH=64.
Outputs: context [B, S, D] and attention probs [H*B, S, S] (heads-major).

Sharding: hybrid batch x head-group. Core c in 0..7 handles batch c//4 and
heads 4*(c%4) .. 4*(c%4)+3.  Each core receives its batch's q/k/v
[S, D] fp32 plus the weight rows for its 4 heads, and produces
  - ctx_out  [S, 256]      (its head-group's slice of context)
  - attn_out [4, S, S]     (its 4 heads' attention matrices)
The host assembles the full outputs.

Per-core pipeline (engines):
  phase 0: cast-load q/k/v to bf16, transpose via PE (matmul vs identity)
           -> qT/kT/vT chunks [128, S] bf16 (d-model on partitions)
  phase 1: projections:  QT/KT [G=256, S] bf16 (head dims on partitions),
           V [S, G] with a ones column per head (65-wide slots) for row sums
  phase 2, per head:
    pass 1 (k-major): scores^T chunk = K @ Q^T on PE (contract=64),
      ACT exp(0.125*x) -> expT bf16; AV matmul accumulates
      lhsT=[V_h | 1] [k,65] x expT -> PSUM [65, S] = [ctx^T ; rowsums].
      ctx^T+sums transposed back via PE; recip/ln via DVE/ACT.
    pass 2 (q-major): scores chunk = Q @ K^T on PE, ACT computes
      exp(0.125*x - ln(rowsum)) (bias is per-partition AP) -> normalized
      softmax fp32 -> DMA out.  (No max-subtraction: scores ~ N(0,1),
      mathematically identical, fp32-safe.)
"""

import os
import sys

import numpy as np

_TRN_REPO = "/opt/trn_rl_repo"
if _TRN_REPO not in sys.path:
    sys.path.insert(0, _TRN_REPO)

# ---- problem constants (hardcoded per contract) ----
B = 2
S = 2048
D_MODEL = 1024
NUM_HEADS = 16
D_HEAD = 64
N_CORES = 8
GROUPS = 4               # head-group splits (cores per batch)
HG = NUM_HEADS // GROUPS  # heads per core = 4
G = HG * D_HEAD           # head-group width = 256


def build_attention_nc(S=S, D=D_MODEL, HG=HG, DH=D_HEAD, chunk=1024):
    """Build the single-core Bass program (SPMD: same program on all cores)."""
    from contextlib import ExitStack

    import concourse.bass as bass
    import concourse.bacc as bacc
    import concourse.mybir as mybir
    from concourse import masks
    from concourse.tile import TileContext

    FP32 = mybir.dt.float32
    BF16 = mybir.dt.bfloat16

    G = HG * DH
    NT = S // 128          # s-tiles
    KC = D // 128          # d-model chunks
    MT = G // 128          # head-group partition tiles
    NS512 = min(512, chunk)
    scale = 1.0 / float(np.sqrt(DH))
    assert S % chunk == 0 and chunk % NS512 == 0

    nc = bacc.Bacc("TRN2", target_bir_lowering=False, debug=False)

    q_in = nc.declare_dram_parameter("q_in", [S, D], FP32, isOutput=False)
    k_in = nc.declare_dram_parameter("k_in", [S, D], FP32, isOutput=False)
    v_in = nc.declare_dram_parameter("v_in", [S, D], FP32, isOutput=False)
    wq = nc.declare_dram_parameter("wq", [G, D], FP32, isOutput=False)
    wk = nc.declare_dram_parameter("wk", [G, D], FP32, isOutput=False)
    wv = nc.declare_dram_parameter("wv", [G, D], FP32, isOutput=False)
    # biases pre-arranged host-side: bq/bk as [128, MT] per-partition columns,
    # bv replicated to [128, G] (tiny constants; layout prep only)
    bq = nc.declare_dram_parameter("bq", [128, MT], FP32, isOutput=False)
    bk = nc.declare_dram_parameter("bk", [128, MT], FP32, isOutput=False)
    bv = nc.declare_dram_parameter("bv", [128, G], FP32, isOutput=False)
    attn_out = nc.declare_dram_parameter("attn_out", [HG, S, S], FP32, isOutput=True)
    ctx_out = nc.declare_dram_parameter("ctx_out", [S, G], FP32, isOutput=True)

    with TileContext(nc) as tc, ExitStack() as ctx:
        singles = ctx.enter_context(tc.tile_pool(name="singles", bufs=1))
        proj_pool = ctx.enter_context(tc.tile_pool(name="proj_pool", bufs=1))
        ps_s = ctx.enter_context(tc.tile_pool(name="ps_s", bufs=2, space="PSUM"))
        ps_av = ctx.enter_context(tc.tile_pool(name="ps_av", bufs=1, space="PSUM"))
        phase01 = ctx.enter_context(ExitStack())
        in_pool = phase01.enter_context(tc.tile_pool(name="in_pool", bufs=3))
        xt_pool = phase01.enter_context(tc.tile_pool(name="xt_pool", bufs=1))

        # ---- constants ----
        ident_bf = singles.tile([128, 128], BF16, name="ident_bf")
        masks.make_identity(nc, ident_bf[:, :])
        ident_f32 = singles.tile([128, 128], FP32, name="ident_f32")
        masks.make_identity(nc, ident_f32[:, :])
        # ---- biases (host pre-arranged layouts) ----
        bq_sb = singles.tile([128, MT], FP32, name="bq_sb")
        bk_sb = singles.tile([128, MT], FP32, name="bk_sb")
        bv_bcast = singles.tile([128, G], FP32, name="bv_bcast")
        nc.sync.dma_start(out=bq_sb[:, :], in_=bq[:, :])
        nc.sync.dma_start(out=bk_sb[:, :], in_=bk[:, :])
        nc.sync.dma_start(out=bv_bcast[:, :], in_=bv[:, :])

        # ---- phase 0: weights: cast-load + PE transpose -> w*T chunks ----
        # w*T[kc] is [128, G] bf16 (d-model chunk on partitions).
        wT = {}
        for wname, wdram in (("wq", wq), ("wk", wk), ("wv", wv)):
            wtiles = []
            for mt in range(MT):
                wt = in_pool.tile([128, D], BF16, tag="wload", name=f"{wname}_ld{mt}")
                nc.gpsimd.dma_start(out=wt[:, :], in_=wdram[mt * 128 : (mt + 1) * 128, :])
                wtiles.append(wt)
            chunks = []
            for kc in range(KC):
                wps = ps_s.tile([128, G], FP32, tag="s", name=f"{wname}_ps{kc}")
                for mt in range(MT):
                    nc.tensor.matmul(
                        wps[:, mt * 128 : (mt + 1) * 128],
                        wtiles[mt][:, kc * 128 : (kc + 1) * 128],
                        ident_bf[:, :],
                        start=True,
                        stop=True,
                    )
                wsb = xt_pool.tile([128, G], BF16, tag=f"wT_{wname}_{kc}",
                                   name=f"{wname}T{kc}")
                nc.vector.tensor_copy(wsb[:, :], wps[:, :])
                chunks.append(wsb)
            wT[wname] = chunks

        # ---- phase 0: inputs: cast-load + PE transpose -> xT chunks ----
        # xT[kc] is [128, S] bf16.  One strided PSUM->SBUF copy per s-tile.
        xT = {}
        for xname, xdram in (("q", q_in), ("k", k_in), ("v", v_in)):
            big = xt_pool.tile([128, KC * S], BF16, tag=f"xT_{xname}", name=f"{xname}T")
            for st in range(NT):
                xt = in_pool.tile([128, D], BF16, tag="xload", name=f"{xname}_ld{st}")
                nc.gpsimd.dma_start(out=xt[:, :], in_=xdram[st * 128 : (st + 1) * 128, :])
                xps = ps_s.tile([128, chunk], FP32, tag="s", name=f"{xname}_ps{st}")
                assert D <= chunk
                for kc in range(KC):
                    nc.tensor.matmul(
                        xps[:, kc * 128 : (kc + 1) * 128],
                        xt[:, kc * 128 : (kc + 1) * 128],
                        ident_bf[:, :],
                        start=True,
                        stop=True,
                    )
                # dest: for each kc, columns [kc*S + st*128, +128)
                dest = big.rearrange("p (kc s) -> p kc s", kc=KC)[
                    :, :, st * 128 : (st + 1) * 128
                ]
                src = xps.rearrange("p (kc c) -> p kc c", kc=KC)
                nc.vector.tensor_copy(dest, src)
            xT[xname] = big.rearrange("p (kc s) -> p kc s", kc=KC)

        # ---- phase 1: projections ----
        # QT/KT: [G, S] bf16 as MT tiles of [128, S]; psum accumulated over KC.
        QT, KT = [], []
        for tname, wch, bias_sb, outl in (("QT", wT["wq"], bq_sb, QT),
                                          ("KT", wT["wk"], bk_sb, KT)):
            for mt in range(MT):
                tsb = proj_pool.tile([128, S], BF16, tag=f"{tname}{mt}",
                                     name=f"{tname}{mt}")
                for nq in range(S // chunk):
                    pps = ps_s.tile([128, chunk], FP32, tag="s",
                                    name=f"{tname}_ps{mt}_{nq}")
                    for kc in range(KC):
                        for ns in range(chunk // NS512):
                            nc.tensor.matmul(
                                pps[:, ns * NS512 : (ns + 1) * NS512],
                                wch[kc][:, mt * 128 : (mt + 1) * 128],
                                xT["q" if tname == "QT" else "k"][
                                    :, kc, nq * chunk + ns * NS512 :
                                    nq * chunk + (ns + 1) * NS512
                                ],
                                start=(kc == 0),
                                stop=(kc == KC - 1),
                            )
                    nc.vector.tensor_scalar(
                        tsb[:, nq * chunk : (nq + 1) * chunk],
                        pps[:, :],
                        bias_sb[:, mt : mt + 1],
                        None,
                        op0=mybir.AluOpType.add,
                    )
                outl.append(tsb)

        # V with ones column: V65[st] is [128, HG*65] bf16; per head h the
        # lhsT slice [:, h*65 : h*65+65] = [V_h (64) | ones (1)].
        V65 = []
        for st in range(NT):
            vps = ps_s.tile([128, G], FP32, tag="s", name=f"v_ps{st}")
            for kc in range(KC):
                nc.tensor.matmul(
                    vps[:, :],
                    xT["v"][:, kc, st * 128 : (st + 1) * 128],
                    wT["wv"][kc][:, :],
                    start=(kc == 0),
                    stop=(kc == KC - 1),
                )
            vsb = proj_pool.tile([128, HG * 65], BF16, tag=f"V65_{st}",
                                 name=f"V65_{st}")
            dest = vsb.rearrange("p (h c) -> p h c", h=HG)[:, :, 0:64]
            src = vps.rearrange("p (h c) -> p h c", h=HG)
            bvv = bv_bcast.rearrange("p (h c) -> p h c", h=HG)
            nc.vector.tensor_tensor(dest, src, bvv, op=mybir.AluOpType.add)
            ones_col = vsb.rearrange("p (h c) -> p h c", h=HG)[:, :, 64:65]
            nc.gpsimd.memset(ones_col, 1.0)
            V65.append(vsb)

        # ---- phase 2: attention per head ----
        phase01.close()  # release input/transpose staging SBUF
        expt_pool = ctx.enter_context(tc.tile_pool(name="expt_pool", bufs=3))
        ctxr_pool = ctx.enter_context(tc.tile_pool(name="ctxr_pool", bufs=2))
        nl_pool = ctx.enter_context(tc.tile_pool(name="nl_pool", bufs=2))
        attn_pool = ctx.enter_context(tc.tile_pool(name="attn_pool", bufs=3))
        ctx_sb = singles.tile([128, NT * G], FP32, name="ctx_sb")

        for p in range(HG):
            mt, off = (p * 64) // 128, (p * 64) % 128
            QhT = QT[mt][off : off + 64, :]
            KhT = KT[mt][off : off + 64, :]

            # ---- pass 1: k-major; scores^T -> exp -> AV accumulate ----
            avp = ps_av.tile([65, S], FP32, tag="av", name=f"av_{p}")
            for kc in range(NT):
                ept = expt_pool.tile([128, S], BF16, tag="expt", name=f"expt_{p}_{kc}")
                for half in range(S // chunk):
                    sps = ps_s.tile([128, chunk], FP32, tag="s",
                                    name=f"sT_{p}_{kc}_{half}")
                    for ns in range(chunk // NS512):
                        nc.tensor.matmul(
                            sps[:, ns * NS512 : (ns + 1) * NS512],
                            KhT[:, kc * 128 : (kc + 1) * 128],
                            QhT[:, half * chunk + ns * NS512 :
                                half * chunk + (ns + 1) * NS512],
                            start=True,
                            stop=True,
                        )
                    nc.scalar.activation(
                        ept[:, half * chunk : (half + 1) * chunk],
                        sps[:, :],
                        mybir.ActivationFunctionType.Exp,
                        scale=scale,
                    )
                for ns4 in range(S // NS512):
                    nc.tensor.matmul(
                        avp[:, ns4 * NS512 : (ns4 + 1) * NS512],
                        V65[kc][:, p * 65 : p * 65 + 65],
                        ept[:, ns4 * NS512 : (ns4 + 1) * NS512],
                        start=(kc == 0),
                        stop=(kc == NT - 1),
                    )

            # ---- ctx^T + sums -> transpose back; ln/recip ----
            ctxT = ctxr_pool.tile([65, S], FP32, tag="ctxT", name=f"ctxT_{p}")
            nc.vector.tensor_copy(ctxT[:, :], avp[:, :])
            craw = ctxr_pool.tile([128, NT * 65], FP32, tag="craw", name=f"craw_{p}")
            for t in range(NT):
                cps = ps_av.tile([128, 65], FP32, tag="av", name=f"cps_{p}_{t}")
                nc.tensor.matmul(
                    cps[:, :],
                    ctxT[:, t * 128 : (t + 1) * 128],
                    ident_f32[0:65, 0:65],
                    start=True,
                    stop=True,
                )
                nc.vector.tensor_copy(craw[:, t * 65 : (t + 1) * 65], cps[:, :])
            sums_view = craw.rearrange("p (t c) -> p t c", t=NT)[:, :, 64]
            lnt = nl_pool.tile([128, NT], FP32, tag="lnt", name=f"lnt_{p}")
            nc.scalar.activation(
                lnt[:, :], sums_view, mybir.ActivationFunctionType.Ln
            )
            negln = nl_pool.tile([128, NT], FP32, tag="negln", name=f"negln_{p}")
            nc.vector.tensor_scalar(
                negln[:, :], lnt[:, :], -1.0, None, op0=mybir.AluOpType.mult
            )
            recip = nl_pool.tile([128, NT], FP32, tag="recip", name=f"recip_{p}")
            nc.scalar.activation(
                recip[:, :], negln[:, :], mybir.ActivationFunctionType.Exp
            )
            ctx_v = ctx_sb.rearrange("p (t g) -> p t g", t=NT)
            craw_v = craw.rearrange("p (t c) -> p t c", t=NT)
            for t in range(NT):
                nc.vector.tensor_scalar(
                    ctx_v[:, t, p * 64 : (p + 1) * 64],
                    craw_v[:, t, 0:64],
                    recip[:, t : t + 1],
                    None,
                    op0=mybir.AluOpType.mult,
                )

            # ---- pass 2: q-major; normalized softmax -> DRAM ----
            QT_PER_TILE = 2
            for qt2 in range(NT // QT_PER_TILE):
                atile = attn_pool.tile([128, QT_PER_TILE * S], FP32, tag="attn",
                                       name=f"attn_{p}_{qt2}")
                for sub in range(QT_PER_TILE):
                    qt = qt2 * QT_PER_TILE + sub
                    for half in range(S // chunk):
                        s2 = ps_s.tile([128, chunk], FP32, tag="s",
                                       name=f"s2_{p}_{qt}_{half}")
                        for ns in range(chunk // NS512):
                            nc.tensor.matmul(
                                s2[:, ns * NS512 : (ns + 1) * NS512],
                                QhT[:, qt * 128 : (qt + 1) * 128],
                                KhT[:, half * chunk + ns * NS512 :
                                    half * chunk + (ns + 1) * NS512],
                                start=True,
                                stop=True,
                            )
                        nc.scalar.activation(
                            atile[:, sub * S + half * chunk :
                                  sub * S + (half + 1) * chunk],
                            s2[:, :],
                            mybir.ActivationFunctionType.Exp,
                            bias=negln[:, qt : qt + 1],
                            scale=scale,
                        )
                dest = attn_out[p,
                                qt2 * QT_PER_TILE * 128 : (qt2 + 1) * QT_PER_TILE * 128,
                                :].rearrange("(sub p) k -> p sub k", sub=QT_PER_TILE)
                nc.sync.dma_start(
                    out=dest,
                    in_=atile.rearrange("p (sub k) -> p sub k", sub=QT_PER_TILE),
                )

        # ---- ctx out: one strided DMA ----
        nc.sync.dma_start(
            out=ctx_out.rearrange("(t p) g -> p t g", p=128),
            in_=ctx_sb.rearrange("p (t g) -> p t g", g=G),
        )

    nc.compile()
    return nc


_NC_CACHE = {}


def _get_nc():
    key = (S, D_MODEL, HG, D_HEAD)
    if key not in _NC_CACHE:
        _NC_CACHE[key] = build_attention_nc()
    return _NC_CACHE[key]


def kernel(query, key, value, Wq, bq, Wk, bk, Wv, bv, _trace=False, _trace_kwargs=None):
    """Full-input, full-output entry point.  Shards across 8 NeuronCores."""
    from concourse.bass_utils import run_bass_kernel_spmd

    query = np.asarray(query, dtype=np.float32)
    key = np.asarray(key, dtype=np.float32)
    value = np.asarray(value, dtype=np.float32)
    Wq = np.asarray(Wq, dtype=np.float32)
    Wk = np.asarray(Wk, dtype=np.float32)
    Wv = np.asarray(Wv, dtype=np.float32)
    bq = np.asarray(bq, dtype=np.float32).reshape(D_MODEL)
    bk = np.asarray(bk, dtype=np.float32).reshape(D_MODEL)
    bv = np.asarray(bv, dtype=np.float32).reshape(D_MODEL)

    nc = _get_nc()
    in_maps = []
    for c in range(N_CORES):
        b, g = c // GROUPS, c % GROUPS
        sl = slice(g * G, (g + 1) * G)
        in_maps.append({
            "q_in": np.ascontiguousarray(query[b]),
            "k_in": np.ascontiguousarray(key[b]),
            "v_in": np.ascontiguousarray(value[b]),
            "wq": np.ascontiguousarray(Wq[sl]),
            "wk": np.ascontiguousarray(Wk[sl]),
            "wv": np.ascontiguousarray(Wv[sl]),
            # [128, MT] per-partition columns / [128, G] broadcast
            "bq": np.ascontiguousarray(bq[sl].reshape(-1, 128).T),
            "bk": np.ascontiguousarray(bk[sl].reshape(-1, 128).T),
            "bv": np.ascontiguousarray(np.tile(bv[sl], (128, 1))),
        })

    kw = dict(_trace_kwargs or {})
    res = run_bass_kernel_spmd(
        nc, in_maps, list(range(N_CORES)), trace=_trace, **kw
    )

    context = np.empty((B, S, D_MODEL), dtype=np.float32)
    attn = np.empty((NUM_HEADS * B, S, S), dtype=np.float32)
    for c in range(N_CORES):
        b, g = c // GROUPS, c % GROUPS
        r = res.results[c]
        context[b][:, g * G : (g + 1) * G] = r["ctx_out"]
        for p in range(HG):
            attn[(HG * g + p) * B + b] = r["attn_out"][p]

    if _trace:
        return (context, attn), res
    return context, attn


# revision 23
# speedup vs baseline: 1.1960x; 1.1960x over previous
"""Multi-head attention Trainium2 kernel (Bass/Tile), 8-core SPMD.

Problem: nn.MultiHeadAttention, B=2, S=2048, D=1024, H=16 heads, DH=64.
Outputs: context [B, S, D] and attention probs [H*B, S, S] (heads-major).

Sharding: hybrid batch x head-group. Core c in 0..7 handles batch c//4 and
heads 4*(c%4) .. 4*(c%4)+3.  Each core receives its batch's q/k/v
[S, D] fp32 plus the weight rows for its 4 heads, and produces
  - ctx_out  [S, 256]      (its head-group's slice of context)
  - attn_out [4, S, S]     (its 4 heads' attention matrices)
The host assembles the full outputs.

Per-core pipeline (engines):
  phase 0: cast-load q/k/v to bf16, transpose via PE (matmul vs identity)
           -> qT/kT/vT chunks [128, S] bf16 (d-model on partitions)
  phase 1: projections:  QT/KT [G=256, S] bf16 (head dims on partitions),
           V [S, G] with a ones column per head (65-wide slots) for row sums
  phase 2, per head:
    pass 1 (k-major): scores^T chunk = K @ Q^T on PE (contract=64),
      ACT exp(0.125*x) -> expT bf16; AV matmul accumulates
      lhsT=[V_h | 1] [k,65] x expT -> PSUM [65, S] = [ctx^T ; rowsums].
      ctx^T+sums transposed back via PE; recip/ln via DVE/ACT.
    pass 2 (q-major): scores chunk = Q @ K^T on PE, ACT computes
      exp(0.125*x - ln(rowsum)) (bias is per-partition AP) -> normalized
      softmax fp32 -> DMA out.  (No max-subtraction: scores ~ N(0,1),
      mathematically identical, fp32-safe.)
"""

import os
import sys

import numpy as np

_TRN_REPO = "/opt/trn_rl_repo"
if _TRN_REPO not in sys.path:
    sys.path.insert(0, _TRN_REPO)

# ---- problem constants (hardcoded per contract) ----
B = 2
S = 2048
D_MODEL = 1024
NUM_HEADS = 16
D_HEAD = 64
N_CORES = 8
GROUPS = 4               # head-group splits (cores per batch)
HG = NUM_HEADS // GROUPS  # heads per core = 4
G = HG * D_HEAD           # head-group width = 256


def build_attention_nc(S=S, D=D_MODEL, HG=HG, DH=D_HEAD, chunk=2048):
    """Build the single-core Bass program (SPMD: same program on all cores)."""
    from contextlib import ExitStack

    import concourse.bass as bass
    import concourse.bacc as bacc
    import concourse.mybir as mybir
    from concourse import masks
    from concourse.tile import TileContext

    FP32 = mybir.dt.float32
    BF16 = mybir.dt.bfloat16

    G = HG * DH
    NT = S // 128          # s-tiles
    KC = D // 128          # d-model chunks
    MT = G // 128          # head-group partition tiles
    chunk = min(chunk, S)
    NS512 = min(512, chunk)
    scale = 1.0 / float(np.sqrt(DH))
    assert S % chunk == 0 and chunk % NS512 == 0

    nc = bacc.Bacc("TRN2", target_bir_lowering=False, debug=False)

    q_in = nc.declare_dram_parameter("q_in", [S, D], FP32, isOutput=False)
    k_in = nc.declare_dram_parameter("k_in", [S, D], FP32, isOutput=False)
    v_in = nc.declare_dram_parameter("v_in", [S, D], FP32, isOutput=False)
    wq = nc.declare_dram_parameter("wq", [G, D], FP32, isOutput=False)
    wk = nc.declare_dram_parameter("wk", [G, D], FP32, isOutput=False)
    wv = nc.declare_dram_parameter("wv", [G, D], FP32, isOutput=False)
    # biases pre-arranged host-side: bq/bk as [128, MT] per-partition columns,
    # bv replicated to [128, G] (tiny constants; layout prep only)
    bq = nc.declare_dram_parameter("bq", [128, MT], FP32, isOutput=False)
    bk = nc.declare_dram_parameter("bk", [128, MT], FP32, isOutput=False)
    bv = nc.declare_dram_parameter("bv", [128, G], FP32, isOutput=False)
    attn_out = nc.declare_dram_parameter("attn_out", [HG, S, S], FP32, isOutput=True)
    ctx_out = nc.declare_dram_parameter("ctx_out", [S, G], FP32, isOutput=True)

    with TileContext(nc) as tc, ExitStack() as ctx:
        singles = ctx.enter_context(tc.tile_pool(name="singles", bufs=1))
        proj_pool = ctx.enter_context(tc.tile_pool(name="proj_pool", bufs=1))
        # Single PSUM pool: 2 slots of [128, chunk] fp32 (4 banks each).
        # Everything (score chunks, AV accumulator, small transposes)
        # time-shares these two slots via tag "s".
        ps_s = ctx.enter_context(tc.tile_pool(name="ps_s", bufs=2, space="PSUM"))
        phase01 = ctx.enter_context(ExitStack())
        in_pool = phase01.enter_context(tc.tile_pool(name="in_pool", bufs=6))
        xt_pool = phase01.enter_context(tc.tile_pool(name="xt_pool", bufs=1))

        # ---- constants ----
        ident_bf = singles.tile([128, 128], BF16, name="ident_bf")
        masks.make_identity(nc, ident_bf[:, :])
        ident_f32 = singles.tile([128, 128], FP32, name="ident_f32")
        masks.make_identity(nc, ident_f32[:, :])
        # ---- biases (host pre-arranged layouts) ----
        bq_sb = singles.tile([128, MT], FP32, name="bq_sb")
        bk_sb = singles.tile([128, MT], FP32, name="bk_sb")
        bv_bcast = singles.tile([128, G], FP32, name="bv_bcast")
        nc.sync.dma_start(out=bq_sb[:, :], in_=bq[:, :])
        nc.sync.dma_start(out=bk_sb[:, :], in_=bk[:, :])
        nc.sync.dma_start(out=bv_bcast[:, :], in_=bv[:, :])

        # ---- phase 0: weights: cast-load + PE transpose -> w*T chunks ----
        # w*T[kc] is [128, G] bf16 (d-model chunk on partitions).
        wT = {}
        for wname, wdram in (("wq", wq), ("wk", wk), ("wv", wv)):
            wtiles = []
            for mt in range(MT):
                wt = in_pool.tile([128, D], BF16, tag="wload", name=f"{wname}_ld{mt}")
                nc.gpsimd.dma_start(out=wt[:, :], in_=wdram[mt * 128 : (mt + 1) * 128, :])
                wtiles.append(wt)
            chunks = []
            for kc in range(KC):
                wps = ps_s.tile([128, G], FP32, tag="s", name=f"{wname}_ps{kc}")
                for mt in range(MT):
                    nc.tensor.matmul(
                        wps[:, mt * 128 : (mt + 1) * 128],
                        wtiles[mt][:, kc * 128 : (kc + 1) * 128],
                        ident_bf[:, :],
                        start=True,
                        stop=True,
                    )
                wsb = xt_pool.tile([128, G], BF16, tag=f"wT_{wname}_{kc}",
                                   name=f"{wname}T{kc}")
                nc.vector.tensor_copy(wsb[:, :], wps[:, :])
                chunks.append(wsb)
            wT[wname] = chunks

        # ---- phase 0: inputs: cast-load + PE transpose -> xT chunks ----
        # xT[kc] is [128, S] bf16.  One strided PSUM->SBUF copy per s-tile.
        xT = {}
        for xname, xdram in (("q", q_in), ("k", k_in), ("v", v_in)):
            big = xt_pool.tile([128, KC * S], BF16, tag=f"xT_{xname}", name=f"{xname}T")
            for st in range(NT):
                xt = in_pool.tile([128, D], BF16, tag="xload", name=f"{xname}_ld{st}")
                nc.gpsimd.dma_start(out=xt[:, :], in_=xdram[st * 128 : (st + 1) * 128, :])
                xps = ps_s.tile([128, chunk], FP32, tag="s", name=f"{xname}_ps{st}")
                assert D <= chunk
                for kc in range(KC):
                    nc.tensor.matmul(
                        xps[:, kc * 128 : (kc + 1) * 128],
                        xt[:, kc * 128 : (kc + 1) * 128],
                        ident_bf[:, :],
                        start=True,
                        stop=True,
                    )
                # dest: for each kc, columns [kc*S + st*128, +128)
                dest = big.rearrange("p (kc s) -> p kc s", kc=KC)[
                    :, :, st * 128 : (st + 1) * 128
                ]
                src = xps[:, 0:D].rearrange("p (kc c) -> p kc c", kc=KC)
                nc.vector.tensor_copy(dest, src)
            xT[xname] = big.rearrange("p (kc s) -> p kc s", kc=KC)

        # ---- phase 1: projections ----
        # QT/KT: [G, S] bf16 as MT tiles of [128, S]; psum accumulated over KC.
        QT, KT = [], []
        for tname, wch, bias_sb, outl in (("QT", wT["wq"], bq_sb, QT),
                                          ("KT", wT["wk"], bk_sb, KT)):
            for mt in range(MT):
                tsb = proj_pool.tile([128, S], BF16, tag=f"{tname}{mt}",
                                     name=f"{tname}{mt}")
                for nq in range(S // chunk):
                    pps = ps_s.tile([128, chunk], FP32, tag="s",
                                    name=f"{tname}_ps{mt}_{nq}")
                    for kc in range(KC):
                        for ns in range(chunk // NS512):
                            nc.tensor.matmul(
                                pps[:, ns * NS512 : (ns + 1) * NS512],
                                wch[kc][:, mt * 128 : (mt + 1) * 128],
                                xT["q" if tname == "QT" else "k"][
                                    :, kc, nq * chunk + ns * NS512 :
                                    nq * chunk + (ns + 1) * NS512
                                ],
                                start=(kc == 0),
                                stop=(kc == KC - 1),
                            )
                    nc.vector.tensor_scalar(
                        tsb[:, nq * chunk : (nq + 1) * chunk],
                        pps[:, :],
                        bias_sb[:, mt : mt + 1],
                        None,
                        op0=mybir.AluOpType.add,
                    )
                outl.append(tsb)

        # V with ones column: V65[st] is [128, HG*65] bf16; per head h the
        # lhsT slice [:, h*65 : h*65+65] = [V_h (64) | ones (1)].
        V65 = []
        for st in range(NT):
            vps = ps_s.tile([128, G], FP32, tag="s", name=f"v_ps{st}")
            for kc in range(KC):
                nc.tensor.matmul(
                    vps[:, :],
                    xT["v"][:, kc, st * 128 : (st + 1) * 128],
                    wT["wv"][kc][:, :],
                    start=(kc == 0),
                    stop=(kc == KC - 1),
                )
            vsb = proj_pool.tile([128, HG * 65], BF16, tag=f"V65_{st}",
                                 name=f"V65_{st}")
            dest = vsb.rearrange("p (h c) -> p h c", h=HG)[:, :, 0:64]
            src = vps.rearrange("p (h c) -> p h c", h=HG)
            bvv = bv_bcast.rearrange("p (h c) -> p h c", h=HG)
            nc.vector.tensor_tensor(dest, src, bvv, op=mybir.AluOpType.add)
            ones_col = vsb.rearrange("p (h c) -> p h c", h=HG)[:, :, 64:65]
            nc.gpsimd.memset(ones_col, 1.0)
            V65.append(vsb)

        # ---- phase 2: attention per head ----
        phase01.close()  # release input/transpose staging SBUF
        # expT stays resident for a whole pair (NT tiles) so the AV matmul
        # can run as one dense PE burst; +2 bufs for cross-pair overlap.
        expt_pool = ctx.enter_context(tc.tile_pool(name="expt_pool", bufs=NT + 2))
        ctxr_pool = ctx.enter_context(tc.tile_pool(name="ctxr_pool", bufs=2))
        nl_pool = ctx.enter_context(tc.tile_pool(name="nl_pool", bufs=2))
        attn_pool = ctx.enter_context(tc.tile_pool(name="attn_pool", bufs=2))
        ctx_sb = singles.tile([128, NT * G], FP32, name="ctx_sb")

        for p in range(HG):
            mt, off = (p * 64) // 128, (p * 64) % 128
            QhT = QT[mt][off : off + 64, :]
            KhT = KT[mt][off : off + 64, :]

            # ---- pass 1: k-major; scores^T -> exp (expT kept resident) ----
            epts = []
            for kc in range(NT):
                ept = expt_pool.tile([128, S], BF16, tag="expt", name=f"expt_{p}_{kc}")
                for half in range(S // chunk):
                    sps = ps_s.tile([128, chunk], FP32, tag="s",
                                    name=f"sT_{p}_{kc}_{half}")
                    for ns in range(chunk // NS512):
                        nc.tensor.matmul(
                            sps[:, ns * NS512 : (ns + 1) * NS512],
                            KhT[:, kc * 128 : (kc + 1) * 128],
                            QhT[:, half * chunk + ns * NS512 :
                                half * chunk + (ns + 1) * NS512],
                            start=True,
                            stop=True,
                        )
                    nc.scalar.activation(
                        ept[:, half * chunk : (half + 1) * chunk],
                        sps[:, :],
                        mybir.ActivationFunctionType.Exp,
                        scale=scale,
                    )
                epts.append(ept)

            # ---- AV: one dense PE burst over resident expT ----
            avp = ps_s.tile([65, S], FP32, tag="s", name=f"av_{p}")
            for ns4 in range(S // NS512):
                for kc in range(NT):
                    nc.tensor.matmul(
                        avp[:, ns4 * NS512 : (ns4 + 1) * NS512],
                        V65[kc][:, p * 65 : p * 65 + 65],
                        epts[kc][:, ns4 * NS512 : (ns4 + 1) * NS512],
                        start=(kc == 0),
                        stop=(kc == NT - 1),
                    )

            # ---- ctx^T + sums -> transpose back; ln/recip ----
            ctxT = ctxr_pool.tile([65, S], FP32, tag="ctxT", name=f"ctxT_{p}")
            nc.vector.tensor_copy(ctxT[:, :], avp[:, :])
            craw = ctxr_pool.tile([128, NT * 65], FP32, tag="craw", name=f"craw_{p}")
            for t in range(NT):
                cps = ps_s.tile([128, 65], FP32, tag="s", name=f"cps_{p}_{t}")
                nc.tensor.matmul(
                    cps[:, :],
                    ctxT[:, t * 128 : (t + 1) * 128],
                    ident_f32[0:65, 0:65],
                    start=True,
                    stop=True,
                )
                nc.vector.tensor_copy(craw[:, t * 65 : (t + 1) * 65], cps[:, :])
            sums_view = craw.rearrange("p (t c) -> p t c", t=NT)[:, :, 64]
            lnt = nl_pool.tile([128, NT], FP32, tag="lnt", name=f"lnt_{p}")
            nc.scalar.activation(
                lnt[:, :], sums_view, mybir.ActivationFunctionType.Ln
            )
            negln = nl_pool.tile([128, NT], FP32, tag="negln", name=f"negln_{p}")
            nc.vector.tensor_scalar(
                negln[:, :], lnt[:, :], -1.0, None, op0=mybir.AluOpType.mult
            )
            recip = nl_pool.tile([128, NT], FP32, tag="recip", name=f"recip_{p}")
            nc.scalar.activation(
                recip[:, :], negln[:, :], mybir.ActivationFunctionType.Exp
            )
            ctx_v = ctx_sb.rearrange("p (t g) -> p t g", t=NT)
            craw_v = craw.rearrange("p (t c) -> p t c", t=NT)
            for t in range(NT):
                nc.vector.tensor_scalar(
                    ctx_v[:, t, p * 64 : (p + 1) * 64],
                    craw_v[:, t, 0:64],
                    recip[:, t : t + 1],
                    None,
                    op0=mybir.AluOpType.mult,
                )

            # ---- pass 2: q-major; normalized softmax -> DRAM ----
            QT_PER_TILE = 2
            for qt2 in range(NT // QT_PER_TILE):
                atile = attn_pool.tile([128, QT_PER_TILE * S], FP32, tag="attn",
                                       name=f"attn_{p}_{qt2}")
                for sub in range(QT_PER_TILE):
                    qt = qt2 * QT_PER_TILE + sub
                    for half in range(S // chunk):
                        s2 = ps_s.tile([128, chunk], FP32, tag="s",
                                       name=f"s2_{p}_{qt}_{half}")
                        for ns in range(chunk // NS512):
                            nc.tensor.matmul(
                                s2[:, ns * NS512 : (ns + 1) * NS512],
                                QhT[:, qt * 128 : (qt + 1) * 128],
                                KhT[:, half * chunk + ns * NS512 :
                                    half * chunk + (ns + 1) * NS512],
                                start=True,
                                stop=True,
                            )
                        nc.scalar.activation(
                            atile[:, sub * S + half * chunk :
                                  sub * S + (half + 1) * chunk],
                            s2[:, :],
                            mybir.ActivationFunctionType.Exp,
                            bias=negln[:, qt : qt + 1],
                            scale=scale,
                        )
                dest = attn_out[p,
                                qt2 * QT_PER_TILE * 128 : (qt2 + 1) * QT_PER_TILE * 128,
                                :].rearrange("(sub p) k -> p sub k", sub=QT_PER_TILE)
                nc.sync.dma_start(
                    out=dest,
                    in_=atile.rearrange("p (sub k) -> p sub k", sub=QT_PER_TILE),
                )

        # ---- ctx out: one strided DMA ----
        nc.sync.dma_start(
            out=ctx_out.rearrange("(t p) g -> p t g", p=128),
            in_=ctx_sb.rearrange("p (t g) -> p t g", g=G),
        )

    nc.compile()
    return nc


_NC_CACHE = {}


def _get_nc():
    key = (S, D_MODEL, HG, D_HEAD)
    if key not in _NC_CACHE:
        _NC_CACHE[key] = build_attention_nc()
    return _NC_CACHE[key]


def kernel(query, key, value, Wq, bq, Wk, bk, Wv, bv, _trace=False, _trace_kwargs=None):
    """Full-input, full-output entry point.  Shards across 8 NeuronCores."""
    from concourse.bass_utils import run_bass_kernel_spmd

    query = np.asarray(query, dtype=np.float32)
    key = np.asarray(key, dtype=np.float32)
    value = np.asarray(value, dtype=np.float32)
    Wq = np.asarray(Wq, dtype=np.float32)
    Wk = np.asarray(Wk, dtype=np.float32)
    Wv = np.asarray(Wv, dtype=np.float32)
    bq = np.asarray(bq, dtype=np.float32).reshape(D_MODEL)
    bk = np.asarray(bk, dtype=np.float32).reshape(D_MODEL)
    bv = np.asarray(bv, dtype=np.float32).reshape(D_MODEL)

    nc = _get_nc()
    in_maps = []
    for c in range(N_CORES):
        b, g = c // GROUPS, c % GROUPS
        sl = slice(g * G, (g + 1) * G)
        in_maps.append({
            "q_in": np.ascontiguousarray(query[b]),
            "k_in": np.ascontiguousarray(key[b]),
            "v_in": np.ascontiguousarray(value[b]),
            "wq": np.ascontiguousarray(Wq[sl]),
            "wk": np.ascontiguousarray(Wk[sl]),
            "wv": np.ascontiguousarray(Wv[sl]),
            # [128, MT] per-partition columns / [128, G] broadcast
            "bq": np.ascontiguousarray(bq[sl].reshape(-1, 128).T),
            "bk": np.ascontiguousarray(bk[sl].reshape(-1, 128).T),
            "bv": np.ascontiguousarray(np.tile(bv[sl], (128, 1))),
        })

    kw = dict(_trace_kwargs or {})
    res = run_bass_kernel_spmd(
        nc, in_maps, list(range(N_CORES)), trace=_trace, **kw
    )

    context = np.empty((B, S, D_MODEL), dtype=np.float32)
    attn = np.empty((NUM_HEADS * B, S, S), dtype=np.float32)
    for c in range(N_CORES):
        b, g = c // GROUPS, c % GROUPS
        r = res.results[c]
        context[b][:, g * G : (g + 1) * G] = r["ctx_out"]
        for p in range(HG):
            attn[(HG * g + p) * B + b] = r["attn_out"][p]

    if _trace:
        return (context, attn), res
    return context, attn
